# revision 21
# baseline (speedup 1.0000x reference)
"""Trainium2 Bass kernel for nn_Attention_55362128445856 (v4).

Dense multi-head attention (B=8, N=1024, C=768, H=12, d=64) with LoRA on the
QKV projection and on the output-projection *output*.

Sharding: pure data-parallel over batch - core b computes batch element b.
Weights are replicated to every core.

v4 design (on top of v3's bf16 + per-head score split):
  - MERGED phase 1/2: q/k production for pair hp+1 is emitted inside pair
    hp's attention loop, so the projection matmuls fill the PE slack under
    the ACT-bound softmax instead of running serially up front (v3 lost
    ~45us to phases executing back-to-back).
  - PSUM budget 3+4+1 = 8 banks exactly:
      * scores: four [128,512] quarter tiles/jt from a 3-bank pool (exp per
        quarter on ACT, WAR at quarter granularity)
      * PV accumulators: 4 banks (v_aug ones-column denominators, M=65)
      * q/k production: ONE bank; each 128-col q/k block is computed as two
        concurrent M=64 col-packed chains (tile_position (0,0)/(0,64))
        sharing the bank, evicted [128,512] at once
  - W_qkv host-shuffled per ct-block to [v(768) | pair0 qk(256) | ...] so
    every weight DMA is a contiguous 512-1536B-per-row chunk.
  - LoRA folded into the weights ON HOST (exact):
      W_qkv_eff = W_qkv + 8*A_qkv@B_qkv
      W_proj_eff = W_proj @ (I + 8*A_proj@B_proj)
      b_eff = b_proj + 8*B_proj^T (A_proj^T b_proj)
"""

import numpy as np
from contextlib import ExitStack

import jax
import concourse.bass as bass
import concourse.bacc as bacc
import concourse.mybir as mybir
import concourse.tile as tile

B, N, C = 8, 1024, 768
H, D = 12, 64
R = 4
LORA_SCALING = 8.0
P = 128
CT = C // P          # 6 contraction tiles over C
TT = N // P          # 8 token tiles of 128
T2 = N // 512        # 2 token tiles of 512
NP = H // 2          # 6 head pairs
F32 = mybir.dt.float32
F32R = mybir.dt.float32r
BF16 = mybir.dt.bfloat16
N_CORES = 8

# shuffled W_qkv column layout per ct block: v block then per-pair q|k
V_OFF = 0            # v columns 0:768
def QK_OFF(hp):      # pair hp: q at QK_OFF, k at QK_OFF+128
    return C + hp * 2 * P


def _pin_act_table():
    """Force every activation onto the one table set that holds Exp, Ln,
    Identity and Copy together, so the Ln/Exp reciprocal never thrashes
    table loads against the softmax Exp (1.28us per reload)."""
    import concourse.bacc as bacc_mod
    import concourse.hw_specs as hw_specs_mod
    if getattr(bacc_mod, "_act_tables_pinned", False):
        return
    orig = hw_specs_mod.get_activation_tables

    def pinned(arch):
        t = orig(arch)
        return {name: (s if name == "natural_log_exp_and_others" else set())
                for name, s in t.items()}

    bacc_mod.get_activation_tables = pinned
    bacc_mod._act_tables_pinned = True


def build_nc(debug=False, repeat=1, phases=(1, 2, 3), ph2_parts="full",
             exp_split=False):
    _pin_act_table()
    nc = bacc.Bacc("TRN2", target_bir_lowering=False, debug=debug,
                   num_devices=N_CORES)

    # x arrives pre-transposed from host: [C, N] feature-major, bf16
    x_d = nc.dram_tensor("x", [C, N], BF16, kind="ExternalInput").ap()
    # W_qkv host-shuffled: per ct row-block, columns [v | q0 k0 | q1 k1 ...]
    wqkv_d = nc.dram_tensor("W_qkv", [C, 3 * C], BF16, kind="ExternalInput").ap()
    wproj_d = nc.dram_tensor("W_proj", [C, C], BF16, kind="ExternalInput").ap()
    bproj_d = nc.dram_tensor("b_proj", [C], F32, kind="ExternalInput").ap()
    # output leaves feature-major: [C, N] bf16
    out_d = nc.dram_tensor("out", [C, N], BF16, kind="ExternalOutput").ap()

    with tile.TileContext(nc) as tc, ExitStack() as ctx:
        const = ctx.enter_context(tc.tile_pool(name="const", bufs=1))

        # gpsimd custom-op library for partition_broadcast (normalization)
        from concourse import library_config
        nc.gpsimd.load_library(library_config.attn)

        ones_f = const.tile([P, H], BF16, tag="ones_f")
        nc.vector.memset(ones_f[:], 1.0)

        # b_eff as [128, 6]: column a holds b_eff[a*128 : (a+1)*128]
        bvec = const.tile([P, CT], F32, tag="bvec")
        nc.sync.dma_start(bvec[:], bproj_d.rearrange("(a p) -> p a", p=P))

        # persistent tiles
        xpool = ctx.enter_context(tc.tile_pool(name="xpool", bufs=1))
        xT = [xpool.tile([P, N], BF16, tag=f"xT{ct}", name=f"xT{ct}")
              for ct in range(CT)]
        wpool = ctx.enter_context(tc.tile_pool(name="wpool", bufs=1))
        w_tiles = [wpool.tile([P, 3 * C], BF16, tag=f"w{ct}", name=f"w{ct}")
                   for ct in range(CT)]
        wppool = ctx.enter_context(tc.tile_pool(name="wppool", bufs=1))
        wp_tiles = [wppool.tile([P, C], BF16, tag=f"wp{ct}", name=f"wp{ct}")
                    for ct in range(CT)]
        qkpool = ctx.enter_context(tc.tile_pool(name="qkpool", bufs=1))
        # qkT[0..5] = q feature-major (head pairs), qkT[6..11] = k
        qkT = [qkpool.tile([P, N], BF16, tag=f"qkT{i}", name=f"qkT{i}")
               for i in range(12)]
        # v token-major, interleaved per head with a ones column:
        # v_all[tt][:, h*65 : h*65+64] = v[tt*128:+128, head h], col h*65+64 = 1
        vpool = ctx.enter_context(tc.tile_pool(name="vpool", bufs=1))
        v_all = [vpool.tile([P, H * (D + 1)], BF16, tag=f"v{t}", name=f"v{t}")
                 for t in range(TT)]
        # attnT aliases the q tiles: qkT[hp] is dead once pair hp's scores
        # are done, exactly when attnT[hp] gets written
        attnT = qkT[:CT]

        # phase-gated builds: init tiles whose producer phase is skipped
        if 1 not in phases or 2 not in phases:
            for i in range(12):
                nc.vector.memset(qkT[i].bitcast(F32)[:], 0.0)
        if 1 not in phases:
            for t in range(TT):
                nc.vector.memset(v_all[t].bitcast(F32)[:], 0.0)
            if 3 in phases:
                for ct in range(CT):
                    nc.vector.memset(wp_tiles[ct].bitcast(F32)[:], 0.0)

        run1 = repeat if 1 in phases else 0
        run2 = repeat if 2 in phases else 0

        # ---------------- DMA issue (order == consumption order) ----------
        if run1:
            for ct in range(CT):
                nc.sync.dma_start(xT[ct][:], x_d[ct * P:(ct + 1) * P, :])
                nc.sync.dma_start(w_tiles[ct][:, V_OFF:V_OFF + C],
                                  wqkv_d[ct * P:(ct + 1) * P, V_OFF:V_OFF + C])
            for hp in range(NP):
                lo = QK_OFF(hp)
                for ct in range(CT):
                    nc.sync.dma_start(
                        w_tiles[ct][:, lo:lo + 2 * P],
                        wqkv_d[ct * P:(ct + 1) * P, lo:lo + 2 * P])
            for ct in range(CT):
                nc.sync.dma_start(wp_tiles[ct][:],
                                  wproj_d[ct * P:(ct + 1) * P, :])

        # ---------------- lead-in: v projection, then merged pair loop ----
        V_JIT = False
        with tc.tile_pool(name="ph2", bufs=1) as ph2:
            if run1:
                for t in range(TT):
                    nc.vector.tensor_copy(
                        v_all[t].rearrange("p (h c) -> p h c",
                                           c=D + 1)[:, :, D],
                        ones_f[:])
            if run1 and not V_JIT:
                with tc.tile_pool(name="pp_v", bufs=2, space="PSUM") as pp_v:
                    for tt in range(TT):
                        lo = tt * P
                        psv = pp_v.tile([P, C], F32, tag="v", name=f"vps{tt}")
                        for off, wd in ((0, 512), (512, 256)):
                            vsl = slice(V_OFF + off, V_OFF + off + wd)
                            for i, ct in enumerate(range(CT)):
                                nc.tensor.matmul(
                                    psv[:, off:off + wd],
                                    xT[ct][:, lo:lo + P],
                                    w_tiles[ct][:, vsl],
                                    start=(i == 0), stop=(i == CT - 1))
                        nc.vector.tensor_copy(
                            v_all[tt].rearrange("p (h c) -> p h c",
                                                c=D + 1)[:, :, 0:D],
                            psv.rearrange("p (h d) -> p h d", d=D))

            actx = ExitStack()
            es = actx.enter_context
            pp_qk = es(tc.tile_pool(name="pp_qk", bufs=1, space="PSUM"))
            pp_sc = es(tc.tile_pool(name="pp_sc", bufs=3, space="PSUM"))
            pp_po = es(tc.tile_pool(name="pp_po", bufs=4, space="PSUM"))

            def emit_qk_round(hp, cp, t2):
                """One q or k 128-col block for 512 tokens: two concurrent
                M=64 col-packed chains into one PSUM bank, evicted at once.
                cp: 0 = q, 1 = k."""
                lo = QK_OFF(hp) + cp * P
                dst = qkT[6 * cp + hp]
                ts2 = slice(t2 * 512, (t2 + 1) * 512)
                ps = pp_qk.tile([P, 512], F32, tag="qkp",
                                name=f"qkp{hp}_{cp}_{t2}")
                for i, ct in enumerate(range(CT)):
                    for half in range(2):
                        hsl = slice(lo + 64 * half, lo + 64 * (half + 1))
                        nc.tensor.matmul(
                            ps[64 * half:64 * (half + 1), :],
                            w_tiles[ct][:, hsl],
                            xT[ct][:, ts2],
                            start=(i == 0), stop=(i == CT - 1),
                            tile_position=(0, 64 * half))
                nc.vector.tensor_copy(dst[:, ts2], ps[:])

            def emit_qk_pair(hp):
                for cp in range(2):
                    for t2 in range(T2):
                        emit_qk_round(hp, cp, t2)

            def emit_v_chain(tt):
                """v for token tile tt, just-in-time inside pair 0: two
                chains [512]+[256] through the score-quarter PSUM pool.
                Chunk boundaries align with whole heads (8x64, 4x64)."""
                lo = tt * P
                for off, wd, h0, hn in ((0, 512, 0, 8), (512, 256, 8, 4)):
                    psv = pp_sc.tile([P, wd], F32, tag="sc",
                                     name=f"vps{tt}_{off}")
                    vsl = slice(V_OFF + off, V_OFF + off + wd)
                    for i, ct in enumerate(range(CT)):
                        nc.tensor.matmul(
                            psv[:], xT[ct][:, lo:lo + P],
                            w_tiles[ct][:, vsl],
                            start=(i == 0), stop=(i == CT - 1))
                    nc.vector.tensor_copy(
                        v_all[tt].rearrange(
                            "p (h c) -> p h c",
                            c=D + 1)[:, h0:h0 + hn, 0:D],
                        psv.rearrange("p (h d) -> p h d", d=D))

            if run1:
                emit_qk_pair(0)

            # ---------------- merged attention + next-pair qk -------------
            for _rep in range(run2):
                for hp in range(NP):
                    qt, kt = qkT[hp], qkT[6 + hp]
                    po = [[pp_po.tile([D + 1, 512], F32, tag="po",
                                      name=f"po{hp}_{hh}_{i2}")
                           for i2 in range(T2)] for hh in range(2)]
                    prs = {}

                    def emit_scores_exp(jt):
                        jsl = slice(jt * P, (jt + 1) * P)
                        qs = {}
                        # issue order A0,B0,A1,B1: rows 0:64 (head A) and
                        # 64:128 (head B) run concurrently on the PE
                        for i2 in range(T2):
                            for hh in range(2):
                                lo = hh * 64
                                sq = pp_sc.tile([P, 512], F32, tag="sc",
                                                name=f"sc{hp}_{jt}_{hh}_{i2}")
                                nc.tensor.matmul(
                                    sq[:],
                                    kt[lo:lo + 64, jsl],
                                    qt[lo:lo + 64,
                                       i2 * 512:(i2 + 1) * 512],
                                    start=True, stop=True)
                                qs[(hh, i2)] = sq
                        pr = {}
                        for i2 in range(T2):
                            for hh in range(2):
                                pq = ph2.tile([P, 512], BF16, tag="pr",
                                              bufs=6,
                                              name=f"pr{hp}_{jt}_{hh}_{i2}")
                                nc.scalar.activation(
                                    pq[:], qs[(hh, i2)][:],
                                    mybir.ActivationFunctionType.Exp,
                                    scale=float(D) ** -0.5)
                                pr[(hh, i2)] = pq
                        prs[jt] = pr

                    def emit_pv(jt):
                        if ph2_parts == "se":
                            return
                        pr = prs.pop(jt)
                        for hh in range(2):
                            h = 2 * hp + hh
                            va = v_all[jt][:, h * (D + 1):(h + 1) * (D + 1)]
                            for i2 in range(T2):
                                nc.tensor.matmul(
                                    po[hh][i2][:], va,
                                    pr[(hh, i2)][:],
                                    start=(jt == 0), stop=(jt == TT - 1))

                    # software pipeline: v (pair 0 only) and next-pair qk
                    # rounds spread between the jt steps
                    emit_scores_exp(0)
                    for jt in range(1, TT):
                        if run1 and V_JIT and hp == 0:
                            emit_v_chain(jt - 1)
                        emit_scores_exp(jt)
                        emit_pv(jt - 1)
                        if run1 and hp + 1 < NP and jt <= 4:
                            emit_qk_round(hp + 1, (jt - 1) // 2, (jt - 1) % 2)
                    if run1 and V_JIT and hp == 0:
                        emit_v_chain(TT - 1)
                    emit_pv(TT - 1)

                    if ph2_parts in ("se", "sepv"):
                        for hh in range(2):
                            for i2 in range(T2):
                                o = ph2.tile([D + 1, 512], BF16, tag="ot",
                                             bufs=4, name=f"ot{hp}_{hh}_{i2}")
                                if ph2_parts == "sepv":
                                    nc.vector.tensor_copy(o[:], po[hh][i2][:])
                                else:
                                    nc.vector.memset(o.bitcast(F32)[:], 0.0)
                        continue

                    # evict PV accumulators as bf16 (den row included), den
                    # rows hop via DMA to pk, 1/den on ACT as exp(-ln(den))
                    ot = [[None] * T2, [None] * T2]
                    pk = ph2.tile([4, 512], BF16, tag="pk", bufs=2,
                                  name=f"pk{hp}")
                    rk = ph2.tile([4, 512], F32, tag="rk", bufs=2,
                                  name=f"rk{hp}")
                    rkb = ph2.tile([4, 512], BF16, tag="rkb", bufs=2,
                                   name=f"rkb{hp}")
                    for hh in range(2):
                        for i2 in range(T2):
                            o = ph2.tile([D + 1, 512], BF16, tag="ot", bufs=4,
                                         name=f"ot{hp}_{hh}_{i2}")
                            nc.vector.tensor_copy(o[:], po[hh][i2][:])
                            ot[hh][i2] = o
                            nc.gpsimd.dma_start(
                                pk[2 * hh + i2:2 * hh + i2 + 1, :],
                                o[64:65, :])
                    nc.scalar.activation(rk[:], pk[:],
                                         mybir.ActivationFunctionType.Ln)
                    nc.scalar.activation(rk[:], rk[:],
                                         mybir.ActivationFunctionType.Exp,
                                         scale=-1.0)
                    # recip bf16-rounded only AFTER exp(-ln(den))
                    nc.vector.tensor_copy(rkb[:], rk[:])
                    for hh in range(2):
                        for i2 in range(T2):
                            isl = slice(i2 * 512, (i2 + 1) * 512)
                            o = ot[hh][i2]
                            r = 2 * hh + i2
                            rc0 = ph2.tile([1, 512], BF16, tag="rc0",
                                           bufs=4, name=f"rc0{hp}_{hh}_{i2}")
                            nc.gpsimd.dma_start(rc0[:], rkb[r:r + 1, :])
                            bcs = ph2.tile([64, 512], BF16, tag="bcs", bufs=2,
                                           name=f"bcs{hp}_{hh}_{i2}")
                            nc.gpsimd.partition_broadcast(bcs[:], rc0[:])
                            if hh == 0:
                                nc.vector.tensor_mul(attnT[hp][0:64, isl],
                                                     o[0:64, :], bcs[:])
                            else:
                                nt = ph2.tile([64, 512], BF16, tag="nt",
                                              bufs=2, name=f"nt{hp}_{i2}")
                                nc.vector.tensor_mul(nt[:], o[0:64, :], bcs[:])
                                nc.gpsimd.dma_start(attnT[hp][64:128, isl],
                                                    nt[:])
            actx.close()

        # ------------- Phase 3: output projection (LoRA pre-folded) -------
        with tc.tile_pool(name="ph3", bufs=1) as ph3, \
             tc.tile_pool(name="pp_y", bufs=3, space="PSUM") as pp_y:
            for _rep in range(repeat if 3 in phases else 0):
                for t2 in range(T2):
                    ts2 = slice(t2 * 512, (t2 + 1) * 512)
                    for cp in range(CT):
                        ps = pp_y.tile([P, 512], F32, tag="y", name=f"y{t2}_{cp}")
                        # contract over early head-pairs first; attnT[4]/[5]
                        # finish last in phase 2, so keep them at chain end
                        order = [((cp + 2 * t2) % 4 + k) % 4
                                 for k in range(4)] + [4, 5]
                        for i, ct in enumerate(order):
                            nc.tensor.matmul(
                                ps[:], wp_tiles[ct][:, cp * P:(cp + 1) * P],
                                attnT[ct][:, ts2],
                                start=(i == 0), stop=(i == CT - 1))
                        st = ph3.tile([P, 512], BF16, tag="st", bufs=4,
                                      name=f"st{t2}_{cp}")
                        # bias add on DVE (per-partition scalar operand)
                        nc.vector.tensor_scalar(
                            st[:], ps[:], bvec[:, cp:cp + 1], None,
                            mybir.AluOpType.add)
                        nc.sync.dma_start(out_d[cp * P:(cp + 1) * P, ts2],
                                          st[:])
    nc.compile()
    return nc


_NC = None
_JITTED = None
_META = None


def _get_nc():
    global _NC
    if _NC is None:
        _NC = build_nc()
    return _NC


def _build_runner():
    global _JITTED, _META
    if _JITTED is not None:
        return
    from jax.experimental.shard_map import shard_map
    from jax.sharding import Mesh, PartitionSpec
    from concourse.bass2jax import (install_neuronx_cc_hook, _bass_exec_p,
                                    partition_id_tensor)

    nc = _get_nc()
    install_neuronx_cc_hook()

    partition_name = (nc.partition_id_tensor.name
                      if nc.partition_id_tensor else None)
    in_names, out_names, out_avals, zero_outs = [], [], [], []
    for alloc in nc.m.functions[0].allocations:
        if not isinstance(alloc, mybir.MemoryLocationSet):
            continue
        name = alloc.memorylocations[0].name
        if alloc.kind == "ExternalInput":
            if name == partition_name:
                continue
            in_names.append(name)
        elif alloc.kind == "ExternalOutput":
            out_names.append(name)
            shape = tuple(alloc.tensor_shape)
            dtype = mybir.dt.np(alloc.dtype)
            out_avals.append(jax.core.ShapedArray(shape, dtype))
            zero_outs.append(np.zeros(shape, dtype))
    n_params = len(in_names)
    all_names = in_names + out_names
    if partition_name is not None:
        all_names = all_names + [partition_name]
    donate = tuple(range(n_params, n_params + len(out_names)))

    def _body(*args):
        operands = list(args)
        if partition_name is not None:
            operands.append(partition_id_tensor())
        outs = _bass_exec_p.bind(
            *operands,
            out_avals=tuple(out_avals),
            in_names=tuple(all_names),
            out_names=tuple(out_names),
            lowering_input_output_aliases=(),
            sim_require_finite=True,
            sim_require_nnan=True,
            nc=nc,
        )
        return tuple(outs)

    devices = jax.devices()[:N_CORES]
    mesh = Mesh(np.asarray(devices), ("core",))
    specs = (PartitionSpec("core"),) * (n_params + len(out_names))
    _JITTED = jax.jit(
        shard_map(_body, mesh=mesh, in_specs=specs,
                  out_specs=(PartitionSpec("core"),) * len(out_names),
                  check_rep=False),
        donate_argnums=donate, keep_unused=True)
    _META = (in_names, out_names, zero_outs)


def make_in_maps(x, W_qkv, W_proj, b_proj, A_qkv, B_qkv, A_proj, B_proj):
    x = np.asarray(x, dtype=np.float32)
    W_qkv = np.asarray(W_qkv, dtype=np.float32)
    W_proj = np.asarray(W_proj, dtype=np.float32)
    b_proj = np.asarray(b_proj, dtype=np.float32)
    A_qkv = np.asarray(A_qkv, dtype=np.float32)
    B_qkv = np.asarray(B_qkv, dtype=np.float32)
    A_proj = np.asarray(A_proj, dtype=np.float32)
    B_proj = np.asarray(B_proj, dtype=np.float32)

    # fold LoRA into the weights (exact algebra, fp32 on host)
    wqkv_eff = W_qkv + LORA_SCALING * (A_qkv @ B_qkv)
    wproj_eff = W_proj + LORA_SCALING * ((W_proj @ A_proj) @ B_proj)
    b_eff = b_proj + LORA_SCALING * (B_proj.T @ (A_proj.T @ b_proj))

    # shuffle W_qkv columns: [v(768) | q0(128) k0(128) | q1 k1 | ...]
    cols = [wqkv_eff[:, 2 * C:3 * C]]
    for hp in range(NP):
        cols.append(wqkv_eff[:, hp * P:(hp + 1) * P])
        cols.append(wqkv_eff[:, C + hp * P:C + (hp + 1) * P])
    wqkv_shuf = np.concatenate(cols, axis=1)

    bf = mybir.dt.np(BF16)
    reps = {
        "W_qkv": np.ascontiguousarray(wqkv_shuf).astype(bf),
        "W_proj": np.ascontiguousarray(wproj_eff).astype(bf),
        "b_proj": np.ascontiguousarray(b_eff),
    }
    return [
        {"x": np.ascontiguousarray(x[b].T).astype(bf), **reps}
        for b in range(N_CORES)
    ]


def kernel(x, W_qkv, W_proj, b_proj, A_qkv, B_qkv, A_proj, B_proj):
    _build_runner()
    in_names, out_names, zero_outs = _META
    in_maps = make_in_maps(x, W_qkv, W_proj, b_proj, A_qkv, B_qkv,
                           A_proj, B_proj)
    per_core = [[np.asarray(m[name]) for name in in_names] for m in in_maps]
    concat_in = [
        np.concatenate([per_core[c][i] for c in range(N_CORES)], axis=0)
        for i in range(len(in_names))
    ]
    concat_zero = [
        np.concatenate([z] * N_CORES, axis=0) for z in zero_outs
    ]
    out_arrs = _JITTED(*concat_in, *concat_zero)
    out = np.asarray(out_arrs[0]).astype(np.float32)  # [8*768, 1024]
    return np.ascontiguousarray(
        out.reshape(B, C, N).transpose(0, 2, 1)).astype(np.float32)
